# revision 15
# baseline (speedup 1.0000x reference)
"""Trainium2 Bass kernel for nn_Critic (LSTM critic over T=512 steps).

Sharding: pure data parallel, 8 cores x 32 batch rows, weights replicated.

Design:
  * Scan matmuls: Ul stationary in fp8e4m3 (x16 pre-scale, un-scaled by the
    gate tanh's free affine), x-projection weights bf16, h carried in bf16.
    fp8/bf16 get the compiler's fast-weight-load and 1 cyc/row moving.
  * sigmoid(x) = (1 + tanh(x/2))/2 algebra folded into pre-scaled weights so
    the scan uses tanh only -- single ACT table set (exp_and_others covers
    Exp+Tanh), no sigmoid set switches. Rescaled state: C == 2c, h' == 2h;
    weight scales absorb all factors at zero runtime cost.
  * Per step the gate pre-activations live in two PSUM banks: [i f g] and
    [o]. One tanh over [i f g] feeds both cell-update operands (u and v);
    o's tanh stays off the critical path until the final h-multiply. The
    x-projection GEMMs write each bank first (start=True clears it), then
    the recurrent Ul matmuls accumulate onto the has_written bits -- the
    x@Wl work never sits on the h-dependent path.
  * xT is b-major (col = b*T + t) so all preamble writes are contiguous;
    the scan's matmul APs absorb the strides.
  * Preamble packs 4 batch rows per PE transpose (action) / 2 rows (osc).

Reference quirks honored (as v1): inp3 = elu(boi) folded into the bias;
osc_state/Woi unused; only osc[..., :64] read.

Math of the rescaling:
  gates: i,f,o = sigma(z) = 0.5*(tanh(z/2)+1); g = tanh(z)
  weights scaled so PSUM holds [z_i/2, z_f/2, z_o/2, z_g]:
    x-side Wl'' = 0.5*Wl (ifo cols), 1.0*Wl (g cols)
    h-side (h' = 2h): Ul'' = 0.25*Ul (ifo), 0.5*Ul (g)
  T = tanh(PSUM)  ->  Ti,Tf,To,Tg
  u  = (Ti+1)*Tg          = 2 i g
  v  = (Tf+1)*C           = 4 f c          (C = 2c)
  C' = 0.5*v + u          = 2(f c + i g)   = 2 c'
  tc = tanh(0.5*C')       = tanh(c')
  h' = (To+1)*tc          = 2 o tanh(c')   = 2h
  out = elu(hmax' @ (Wo/2) + bo)
"""

import os
import sys

sys.path.insert(0, "/opt/trn_rl_repo")

from contextlib import ExitStack

import numpy as np

import concourse.bass as bass
import concourse.bacc as bacc
import concourse.mybir as mybir
import concourse.tile as tile
from concourse.masks import make_identity

FP32 = mybir.dt.float32
BF16 = mybir.dt.bfloat16
FP8 = mybir.dt.float8e4
AF = mybir.ActivationFunctionType
ALU = mybir.AluOpType

# Ul (and the x-projection, to keep one uniform PSUM scale) can be carried in
# fp8e4m3: weights are pre-scaled by 16 so they sit in e4m3's normal range,
# and the gate tanh un-scales via its free affine (scale=1/16). Halves the
# per-step LDWEIGHTS cost on hardware (FWL reads 4 fp8/cycle vs 2 bf16).
UL_FP8 = not bool(os.environ.get("KERNEL_UL_BF16"))
ZSCALE = 16.0 if UL_FP8 else 1.0

B_FULL, T_FULL, A = 256, 512, 32
DM, DR = 64, 128
U = 256
OSC_HALF = 64
NCORES = 8
B = B_FULL // NCORES    # 32
XR = A + OSC_HALF       # 96 data rows of xT (rows 0:64 inp2, 64:96 act), row 96 ones
# psum gate-block beta -> source 128-col chunk of [Ul | Wl] (col order i f g o)
# z block layout: bank 1 = [i0 i1 f0 f1 g0 g1] (one tanh feeds both u and v),
# bank 2 = [o0 o1] (only needed for the final h-multiply)
SRC_CHUNK = [0, 1, 2, 3, 4, 5, 6, 7]


def _elu(nc, pool, out_ap, y_ap, shape):
    """out = elu(y) = max(y, exp(min(y, 0)) - 1), exact."""
    m = pool.tile(shape, FP32, tag="elu_m")
    nc.vector.tensor_scalar_min(m, y_ap, 0.0)
    e = pool.tile(shape, FP32, tag="elu_e")
    nc.scalar.activation(e, m, AF.Exp)
    nc.vector.scalar_tensor_tensor(out_ap, e, -1.0, y_ap, ALU.add, ALU.max)


def build_nc(T=T_FULL):
    nc = bacc.Bacc("TRN2", target_bir_lowering=False, debug=False)

    d_action = nc.dram_tensor("action", [B, T, A], FP32, kind="ExternalInput").ap()
    d_osc = nc.dram_tensor("osc", [B, T, OSC_HALF], FP32, kind="ExternalInput").ap()
    d_motion = nc.dram_tensor("motion_state", [B, DM], FP32, kind="ExternalInput").ap()
    d_robot = nc.dram_tensor("robot_state", [B, DR], FP32, kind="ExternalInput").ap()
    d_mu = nc.dram_tensor("mu", [B, A], FP32, kind="ExternalInput").ap()
    d_mean = nc.dram_tensor("mean", [B, A], FP32, kind="ExternalInput").ap()
    d_Wm = nc.dram_tensor("Wm", [DM, U], FP32, kind="ExternalInput").ap()
    d_bm = nc.dram_tensor("bm", [U], FP32, kind="ExternalInput").ap()
    d_Wr = nc.dram_tensor("Wr", [DR, U], FP32, kind="ExternalInput").ap()
    d_br = nc.dram_tensor("br", [U], FP32, kind="ExternalInput").ap()
    d_Wc = nc.dram_tensor("Wc", [2 * U, U], FP32, kind="ExternalInput").ap()
    d_bc = nc.dram_tensor("bc", [U], FP32, kind="ExternalInput").ap()
    d_Wor = nc.dram_tensor("Wor", [OSC_HALF, OSC_HALF], FP32, kind="ExternalInput").ap()
    d_bor = nc.dram_tensor("bor", [OSC_HALF], FP32, kind="ExternalInput").ap()
    d_boi = nc.dram_tensor("boi", [OSC_HALF], FP32, kind="ExternalInput").ap()
    d_Wl = nc.dram_tensor("Wl", [A + 2 * OSC_HALF, 4 * U], FP32, kind="ExternalInput").ap()
    d_bl = nc.dram_tensor("bl", [4 * U], FP32, kind="ExternalInput").ap()
    d_Ul = nc.dram_tensor("Ul", [U, 4 * U], FP32, kind="ExternalInput").ap()
    d_Wo = nc.dram_tensor("Wo", [U, 1], FP32, kind="ExternalInput").ap()
    d_bo = nc.dram_tensor("bo", [1], FP32, kind="ExternalInput").ap()
    d_out = nc.dram_tensor("out", [B, 1], FP32, kind="ExternalOutput").ap()

    with tile.TileContext(nc) as tc, ExitStack() as ctx:
        _build_body(ctx, tc, T,
                    d_action, d_osc, d_motion, d_robot, d_mu, d_mean,
                    d_Wm, d_bm, d_Wr, d_br, d_Wc, d_bc, d_Wor, d_bor, d_boi,
                    d_Wl, d_bl, d_Ul, d_Wo, d_bo, d_out)
    nc.finalize()
    return nc


def _build_body(ctx, tc, T,
                d_action, d_osc, d_motion, d_robot, d_mu, d_mean,
                d_Wm, d_bm, d_Wr, d_br, d_Wc, d_bc, d_Wor, d_bor, d_boi,
                d_Wl, d_bl, d_Ul, d_Wo, d_bo, d_out):
    nc = tc.nc
    SKIP_PRE = bool(os.environ.get("KERNEL_SKIP_PRE"))
    SKIP_SCAN = bool(os.environ.get("KERNEL_SKIP_SCAN"))

    consts = ctx.enter_context(tc.tile_pool(name="consts", bufs=1))
    weights = ctx.enter_context(tc.tile_pool(name="weights", bufs=1))
    state = ctx.enter_context(tc.tile_pool(name="state", bufs=1))
    scratch = ctx.enter_context(tc.tile_pool(name="scratch", bufs=3))

    ident = consts.tile([128, 128], FP32)
    make_identity(nc, ident)
    ones_rb = consts.tile([1, B], BF16)
    nc.vector.memset(ones_rb, 1.0)

    # =================== weight prep (fp32 -> scaled bf16) ===================
    ul_sc = [weights.tile([128, 4 * U], FP8 if UL_FP8 else BF16,
                          tag=f"ulsc{k}", name=f"ulsc{k}")
             for k in range(2)]
    wl_sc = weights.tile([XR + 1, 4 * U], BF16)
    with tc.tile_pool(name="wstage", bufs=1) as wstage:
        ul_st = [wstage.tile([128, 4 * U], FP32, tag=f"ul_st{k}", name=f"ul_st{k}")
                 for k in range(2)]
        for k in range(2):
            nc.sync.dma_start(out=ul_st[k], in_=d_Ul[128 * k:128 * (k + 1), :])
        for k in range(2):
            nc.vector.tensor_scalar_mul(ul_sc[k][:, 0:512], ul_st[k][:, 0:512],
                                        0.25 * ZSCALE)
            nc.vector.tensor_scalar_mul(ul_sc[k][:, 512:768], ul_st[k][:, 512:768],
                                        0.5 * ZSCALE)
            nc.vector.tensor_scalar_mul(ul_sc[k][:, 768:1024], ul_st[k][:, 768:1024],
                                        0.25 * ZSCALE)

        # Wl rows swapped: [inp2(64); act(32)]; bias row 96 = blEff
        wl_st = wstage.tile([XR, 4 * U], FP32, tag="wl_st")
        nc.sync.dma_start(out=wl_st[0:OSC_HALF, :], in_=d_Wl[A:A + OSC_HALF, :])
        nc.sync.dma_start(out=wl_st[OSC_HALF:XR, :], in_=d_Wl[0:A, :])
        nc.vector.tensor_scalar_mul(wl_sc[0:XR, 0:512], wl_st[:, 0:512], 0.5 * ZSCALE)
        nc.vector.tensor_scalar_mul(wl_sc[0:XR, 512:768], wl_st[:, 512:768], 1.0 * ZSCALE)
        nc.vector.tensor_scalar_mul(wl_sc[0:XR, 768:1024], wl_st[:, 768:1024], 0.5 * ZSCALE)

        # blEff = bl + elu(boi) @ Wl[96:160, :]
        boi_sb = scratch.tile([OSC_HALF, 1], FP32)
        nc.sync.dma_start(out=boi_sb, in_=d_boi.rearrange("(p one) -> p one", one=1))
        eboi = scratch.tile([OSC_HALF, 1], FP32)
        _elu(nc, scratch, eboi, boi_sb, [OSC_HALF, 1])
        wl_hi = wstage.tile([OSC_HALF, 4 * U], FP32, tag="wl_hi")
        nc.sync.dma_start(out=wl_hi, in_=d_Wl[XR:XR + OSC_HALF, :])
        with tc.tile_pool(name="pbl", bufs=1, space="PSUM") as pbl_pool:
            p_bl = pbl_pool.tile([1, 4 * U], FP32)
            for half in range(2):
                nc.tensor.matmul(p_bl[:, 512 * half:512 * (half + 1)],
                                 eboi, wl_hi[:, 512 * half:512 * (half + 1)],
                                 start=True, stop=True)
            bl_sb = scratch.tile([1, 4 * U], FP32)
            nc.sync.dma_start(out=bl_sb, in_=d_bl.rearrange("(one n) -> one n", one=1))
            bleff = scratch.tile([1, 4 * U], FP32)
            nc.vector.tensor_add(bleff, p_bl, bl_sb)
        nc.vector.tensor_scalar_mul(wl_sc[XR:XR + 1, 0:512], bleff[:, 0:512],
                                    0.5 * ZSCALE)
        nc.vector.tensor_scalar_mul(wl_sc[XR:XR + 1, 512:768], bleff[:, 512:768],
                                    1.0 * ZSCALE)
        nc.vector.tensor_scalar_mul(wl_sc[XR:XR + 1, 768:1024], bleff[:, 768:1024],
                                    0.5 * ZSCALE)

    worb = weights.tile([OSC_HALF + 1, OSC_HALF], FP32)
    nc.sync.dma_start(out=worb[0:OSC_HALF, :], in_=d_Wor)
    nc.sync.dma_start(out=worb[OSC_HALF:OSC_HALF + 1, :],
                      in_=d_bor.rearrange("(one n) -> one n", one=1))

    wmb = [weights.tile([DM + 1, 128], FP32, tag=f"wm{c}", name=f"wm{c}") for c in range(2)]
    for c in range(2):
        nc.sync.dma_start(out=wmb[c][0:DM, :], in_=d_Wm[:, 128 * c:128 * (c + 1)])
        nc.sync.dma_start(out=wmb[c][DM:DM + 1, :],
                          in_=d_bm.rearrange("(one n) -> one n", one=1)[:, 128 * c:128 * (c + 1)])
    wrb = [weights.tile([DR, 128], FP32, tag=f"wr{c}", name=f"wr{c}") for c in range(2)]
    brb = [weights.tile([1, 128], FP32, tag=f"br{c}", name=f"br{c}") for c in range(2)]
    for c in range(2):
        nc.sync.dma_start(out=wrb[c], in_=d_Wr[:, 128 * c:128 * (c + 1)])
        nc.sync.dma_start(out=brb[c],
                          in_=d_br.rearrange("(one n) -> one n", one=1)[:, 128 * c:128 * (c + 1)])
    wcb = [[weights.tile([128, 128], FP32, tag=f"wc{k}{c}", name=f"wc{k}{c}") for c in range(2)]
           for k in range(4)]
    bcb = [weights.tile([1, 128], FP32, tag=f"bc{c}", name=f"bc{c}") for c in range(2)]
    for k in range(4):
        for c in range(2):
            nc.sync.dma_start(out=wcb[k][c],
                              in_=d_Wc[128 * k:128 * (k + 1), 128 * c:128 * (c + 1)])
    for c in range(2):
        nc.sync.dma_start(out=bcb[c],
                          in_=d_bc.rearrange("(one n) -> one n", one=1)[:, 128 * c:128 * (c + 1)])

    wob = [weights.tile([128, 1], BF16, tag=f"wo{c}", name=f"wo{c}") for c in range(2)]
    bob = weights.tile([1, 1], BF16)
    wo_st = scratch.tile([128, 2], FP32)
    nc.sync.dma_start(out=wo_st, in_=d_Wo.rearrange("(c p) one -> p (c one)", c=2))
    for c in range(2):
        nc.vector.tensor_scalar_mul(wob[c], wo_st[:, c:c + 1], 0.5)
    bo_st = scratch.tile([1, 1], FP32)
    nc.sync.dma_start(out=bo_st, in_=d_bo.rearrange("(one n) -> one n", one=1))
    nc.vector.tensor_copy(bob, bo_st)

    # =================== xT: [97, B*T] bf16, col = b*T + t ==================
    xT = state.tile([XR + 1, B * T], BF16)
    nc.vector.memset(xT[XR:XR + 1, :], 1.0)
    if SKIP_PRE:
        nc.vector.memset(xT[0:XR, :], 0.01)

    TCH = T // 128 if T % 128 == 0 else 0  # 128-step transpose chunks

    with tc.tile_pool(name="ptrans", bufs=2, space="PSUM") as ptrans, \
         tc.tile_pool(name="pmm", bufs=2, space="PSUM") as pmm, \
         tc.tile_pool(name="stg", bufs=3) as stg:

        # ---- mu/mean transposes: [B, A] -> [A, B] fp32 ----
        mu_sb = scratch.tile([B, A], FP32)
        mean_sb = scratch.tile([B, A], FP32)
        nc.sync.dma_start(out=mu_sb, in_=d_mu)
        nc.sync.dma_start(out=mean_sb, in_=d_mean)
        muT = consts.tile([A, B], FP32)
        meanT = consts.tile([A, B], FP32)
        for src, dst in ((mu_sb, muT), (mean_sb, meanT)):
            pt = ptrans.tile([A, B], FP32, tag="pt", name="pt_mu")
            nc.tensor.transpose(pt, src, ident[0:B, 0:B])
            nc.vector.tensor_copy(dst, pt)

        if not SKIP_PRE:
            assert TCH > 0, "T must be a multiple of 128 for the preamble"
            # ---- action: 4 batch rows per [128,128] transpose ----
            for j in range(TCH):
                for g in range(B // 4):
                    a_tile = stg.tile([128, 128], FP32, tag="a_in")
                    nc.sync.dma_start(
                        out=a_tile.rearrange("p (b a) -> p b a", a=A),
                        in_=d_action.rearrange("b t a -> t b a")[
                            128 * j:128 * (j + 1), 4 * g:4 * (g + 1), :])
                    pt = ptrans.tile([128, 128], FP32, tag="pt", name="pt_a")
                    nc.tensor.transpose(pt, a_tile, ident)
                    for bp in range(4):
                        b = 4 * g + bp
                        nc.vector.tensor_scalar(
                            xT[OSC_HALF:XR, T * b + 128 * j:T * b + 128 * (j + 1)],
                            pt[32 * bp:32 * (bp + 1), :],
                            muT[:, b:b + 1], meanT[:, b:b + 1],
                            ALU.mult, ALU.add)

            # ---- osc: 2 batch rows per [128,128] transpose -> oscT fp32 ----
            oscT = state.tile([OSC_HALF + 1, B * T], FP32)
            nc.vector.memset(oscT[OSC_HALF:OSC_HALF + 1, :], 1.0)
            for j in range(TCH):
                for g in range(B // 2):
                    o_tile = stg.tile([128, 128], FP32, tag="o_in")
                    nc.sync.dma_start(
                        out=o_tile.rearrange("p (b o) -> p b o", o=OSC_HALF),
                        in_=d_osc.rearrange("b t o -> t b o")[
                            128 * j:128 * (j + 1), 2 * g:2 * (g + 1), :])
                    pt = ptrans.tile([128, 128], FP32, tag="pt", name="pt_o")
                    nc.tensor.transpose(pt, o_tile, ident)
                    for bp in range(2):
                        b = 2 * g + bp
                        nc.vector.tensor_copy(
                            oscT[0:OSC_HALF, T * b + 128 * j:T * b + 128 * (j + 1)],
                            pt[64 * bp:64 * (bp + 1), :])

            # ---- inp2 = elu(osc @ Wor + bor) -> xT[0:64] ----
            # t-block-outer (j) so the scan's step-t x-matmuls unblock after
            # the first 128-step block instead of after the whole osc pipeline
            xTi3 = xT[0:OSC_HALF, :].rearrange("p (b t) -> p b t", t=T)
            for j in range(TCH):
                for g in range(B // 8):
                    pw = pmm.tile([OSC_HALF, 1024], FP32, tag="pw", name="pw")
                    for bb in range(8):
                        b = 8 * g + bb
                        nc.tensor.matmul(
                            pw[:, 128 * bb:128 * (bb + 1)], worb,
                            oscT[:, T * b + 128 * j:T * b + 128 * (j + 1)],
                            start=True, stop=True)
                    pw3 = pw.rearrange("p (b t) -> p b t", t=128)
                    m_ = scratch.tile([OSC_HALF, 1024], FP32, tag="elu_m")
                    nc.vector.tensor_scalar_min(m_, pw, 0.0)
                    e_ = scratch.tile([OSC_HALF, 1024], FP32, tag="elu_e")
                    nc.scalar.activation(e_, m_, AF.Exp)
                    dst3 = xTi3[:, 8 * g:8 * (g + 1), 128 * j:128 * (j + 1)]
                    nc.vector.scalar_tensor_tensor(
                        dst3, e_.rearrange("p (b t) -> p b t", t=128), -1.0,
                        pw3, ALU.add, ALU.max)

        # ---- h0 = c0 chain ----
        motT = scratch.tile([DM + 1, B], FP32)
        mot_sb = scratch.tile([B, DM], FP32)
        nc.sync.dma_start(out=mot_sb, in_=d_motion)
        pt = ptrans.tile([DM, B], FP32, tag="pt", name="pt_mot")
        nc.tensor.transpose(pt, mot_sb, ident[0:B, 0:B])
        nc.vector.tensor_copy(motT[0:DM, :], pt)
        nc.vector.memset(motT[DM:DM + 1, :], 1.0)

        robT = scratch.tile([DR, B], FP32)
        rob_sb = scratch.tile([B, DR], FP32)
        nc.sync.dma_start(out=rob_sb, in_=d_robot)
        pt = ptrans.tile([DR, B], FP32, tag="pt", name="pt_rob")
        nc.tensor.transpose(pt, rob_sb, ident[0:B, 0:B])
        nc.vector.tensor_copy(robT, pt)

        ones_rf = consts.tile([1, B], FP32)
        nc.vector.memset(ones_rf, 1.0)

        p_ms = pmm.tile([128, 2 * B], FP32, tag="pw", name="p_ms")
        for c in range(2):
            nc.tensor.matmul(p_ms[:, B * c:B * (c + 1)], wmb[c], motT,
                             start=True, stop=True)
        msT = scratch.tile([128, 2 * B], FP32, tag="msT")
        _elu(nc, scratch, msT, p_ms, [128, 2 * B])

        p_rs = pmm.tile([128, 2 * B], FP32, tag="pw", name="p_rs")
        for c in range(2):
            sl = p_rs[:, B * c:B * (c + 1)]
            nc.tensor.matmul(sl, wrb[c], robT, start=True, stop=False)
            nc.tensor.matmul(sl, brb[c], ones_rf, start=False, stop=True)
        rsT = scratch.tile([128, 2 * B], FP32, tag="rsT")
        _elu(nc, scratch, rsT, p_rs, [128, 2 * B])

        p_st = pmm.tile([128, 2 * B], FP32, tag="pw", name="p_st")
        for c in range(2):
            sl = p_st[:, B * c:B * (c + 1)]
            nc.tensor.matmul(sl, wcb[0][c], msT[:, 0:B], start=True, stop=False)
            nc.tensor.matmul(sl, wcb[1][c], msT[:, B:2 * B], start=False, stop=False)
            nc.tensor.matmul(sl, wcb[2][c], rsT[:, 0:B], start=False, stop=False)
            nc.tensor.matmul(sl, wcb[3][c], rsT[:, B:2 * B], start=False, stop=False)
            nc.tensor.matmul(sl, bcb[c], ones_rf, start=False, stop=True)

        h0f = scratch.tile([128, 2 * B], FP32, tag="h0f")
        _elu(nc, scratch, h0f, p_st, [128, 2 * B])
        h = state.tile([128, 2 * B], BF16)
        C = state.tile([128, 2 * B], FP32)
        nc.vector.tensor_scalar_mul(h, h0f, 2.0)
        nc.vector.tensor_scalar_mul(C, h0f, 2.0)

    hmax = state.tile([128, 2 * B], BF16)
    nc.vector.memset(hmax, -1e30)

    # ============================ the scan ==================================
    # Per step, two PSUM banks: bank A = [i0 i1 g0 g1], bank B = [f0 f1 o0 o1]
    # so tanh(A) starts after only the 8 A-matmuls (bank-level collision rule).
    gates = ctx.enter_context(tc.tile_pool(name="gates", bufs=3))
    pz_warm = ctx.enter_context(tc.tile_pool(name="pzwarm", bufs=1, space="PSUM"))
    pz_pool = ctx.enter_context(tc.tile_pool(name="pz", bufs=2, space="PSUM"))
    pz_poolB = ctx.enter_context(tc.tile_pool(name="pzB", bufs=2, space="PSUM"))
    xTb = xT.rearrange("p (b t) -> p b t", t=T)
    T_SCAN = 0 if SKIP_SCAN else T
    REPS = int(os.environ.get("KERNEL_SCAN_REPS", "1"))
    for _rep in range(REPS):
        for t in range(T_SCAN):
            pz1 = pz_pool.tile([128, 192], FP32, tag="pz1", name="pz1")
            pz2 = pz_poolB.tile([128, 64], FP32, tag="pz2", name="pz2")
            # x-projection (independent of h)
            for beta in range(8):
                m = SRC_CHUNK[beta]
                pz, bb = (pz1, beta) if beta < 6 else (pz2, beta - 6)
                nc.tensor.matmul(
                    pz[:, 32 * bb:32 * (bb + 1)],
                    wl_sc[:, 128 * m:128 * (m + 1)], xTb[:, :, t],
                    start=(bb == 0), stop=False, skip_group_check=True)
            # recurrent matmuls: i,f,g blocks first, o last
            for beta in range(8):
                m = SRC_CHUNK[beta]
                pz, bb = (pz1, beta) if beta < 6 else (pz2, beta - 6)
                sl = pz[:, 32 * bb:32 * (bb + 1)]
                nc.tensor.matmul(sl, ul_sc[0][:, 128 * m:128 * (m + 1)],
                                 h[:, 0:B], start=False, stop=False,
                                 skip_group_check=True)
                nc.tensor.matmul(sl, ul_sc[1][:, 128 * m:128 * (m + 1)],
                                 h[:, B:2 * B], start=False,
                                 stop=(beta in (5, 7)), skip_group_check=True)
            if t % 1 == 0:
                # PE warmers: keep the HAM clock-gate at 8/8 through the tail
                # wait (junk matmuls into a scratch bank, never read)
                jw = pz_warm.tile([128, 512], FP32, tag="jw", name="jw")
                for _w in range(6):
                    nc.tensor.matmul(jw, wl_sc[0:97, 0:128],
                                     xT[:, 0:512],
                                     start=True, stop=True,
                                     skip_group_check=True)
            T1 = gates.tile([128, 192], BF16, tag="T1")   # [Ti | Tf | Tg]
            nc.scalar.activation(T1, pz1, AF.Tanh, scale=1.0 / ZSCALE)
            v = gates.tile([128, 64], FP32, tag="v")
            nc.vector.scalar_tensor_tensor(v, T1[:, 64:128], 1.0, C,
                                           ALU.add, ALU.mult)
            u = gates.tile([128, 64], BF16, tag="u")
            nc.vector.scalar_tensor_tensor(u, T1[:, 0:64], 1.0, T1[:, 128:192],
                                           ALU.add, ALU.mult)
            nc.vector.scalar_tensor_tensor(C, v, 0.5, u, ALU.mult, ALU.add)
            T2 = gates.tile([128, 64], BF16, tag="T2")    # [To]
            nc.scalar.activation(T2, pz2, AF.Tanh, scale=1.0 / ZSCALE)
            tc_ = gates.tile([128, 64], BF16, tag="tc")
            nc.scalar.activation(tc_, C, AF.Tanh, scale=0.5)
            nc.vector.scalar_tensor_tensor(h, T2, 1.0, tc_,
                                           ALU.add, ALU.mult)
            nc.vector.tensor_max(hmax, hmax, h)

    # ============================ output ====================================
    with tc.tile_pool(name="pout", bufs=1, space="PSUM") as pout_pool:
        p_out = pout_pool.tile([1, B], FP32)
        nc.tensor.matmul(p_out, wob[0], hmax[:, 0:B], start=True, stop=False)
        nc.tensor.matmul(p_out, wob[1], hmax[:, B:2 * B], start=False, stop=False)
        nc.tensor.matmul(p_out, bob, ones_rb, start=False, stop=True)
        out_sb = scratch.tile([1, B], FP32)
        _elu(nc, scratch, out_sb, p_out, [1, B])
        nc.sync.dma_start(out=d_out.rearrange("b one -> one b"), in_=out_sb)


# ------------------------------------------------------------------
# host-side entry point
# ------------------------------------------------------------------
_CACHE = {}


def _shard_inputs(inputs, T):
    batch_keys = ["action", "osc", "motion_state", "robot_state", "mu", "mean"]
    wkeys = ["Wm", "bm", "Wr", "br", "Wc", "bc", "Wor", "bor", "boi",
             "Wl", "bl", "Ul", "Wo", "bo"]
    in_maps = []
    for i in range(NCORES):
        s = slice(B * i, B * (i + 1))
        m = {}
        for k in batch_keys:
            v = np.asarray(inputs[k], dtype=np.float32)[s]
            if k == "action":
                v = v[:, :T]
            elif k == "osc":
                v = v[:, :T, :OSC_HALF]
            m[k] = np.ascontiguousarray(v)
        for k in wkeys:
            m[k] = np.ascontiguousarray(np.asarray(inputs[k], dtype=np.float32))
        in_maps.append(m)
    return in_maps


def kernel(**inputs) -> np.ndarray:
    from concourse.bass_utils import run_bass_kernel_spmd

    T = int(np.asarray(inputs["action"]).shape[1])
    if T not in _CACHE:
        _CACHE[T] = build_nc(T)
    nc = _CACHE[T]
    in_maps = _shard_inputs(inputs, T)
    res = run_bass_kernel_spmd(nc, in_maps, list(range(NCORES)))
    out = np.concatenate([res.results[i]["out"] for i in range(NCORES)], axis=0)
    return out.astype(np.float32)


if __name__ == "__main__":
    nc = build_nc(128)
    print("built ok")
